# revision 22
# baseline (speedup 1.0000x reference)
"""Trainium2 Bass kernel for ActorMoE (8 experts, dims 512->1024->512->256->64).

Strategy: data-parallel across 8 NeuronCores (2048 rows each), weights
replicated. Feature-major on device (features on partitions, batch on the
free dim) so stacked expert weights W_l[e] ([in, out]) are directly the
matmul lhsT.

The expert loop is a HARDWARE For_i loop (not unrolled): the PE sequencer
re-executes a ~600-instruction body, which stays inside the instruction
cache. A fully unrolled 8-expert stream (~4800 PE instructions) runs at
~2.5x lower issue rate (instruction-fetch bound) — measured 230+ ns per
matmul vs ~90 ns for cache-resident loops of the same shape.

Weights live in single-buffered static SBUF tiles (ldweights cannot take
register offsets); expert e+1's tiles are DMA-prefetched mid-iteration
with dynamic (register-offset) DRAM sources right after layer l of expert
e releases them. WA/WB/BIAS are padded with a 9th zero expert so the
prefetch at e=7 stays in bounds.

ELU: h' = elu(z+b)+1 = min(exp(z+b), 1) + relu(z+b), computed as two ACT
passes over PSUM (exp is ~1 Telem/s on ACT) and one all-bf16 DVE
scalar_tensor_tensor merge: (e min 1.0) add r. The +1 shift is corrected
by subtracting colsum(W_next) from the next layer's bias on the host.

Gate (prologue): softmax weights w[e, token] -> wT (bf16) -> bounced to
DRAM. Per expert iteration, w[e] is row-broadcast-DMA'd over 128
partitions, h3 is pre-scaled by it (DVE), and L3 matmuls accumulate into
PSUM; acc(sbuf f32) += psum. acc is initialised with sum_e w[e]*b3_e via
one tiny matmul (lhsT = stacked b3, rhs = wT) before the loop.
"""

import sys

sys.path.insert(0, "/opt/trn_rl_repo")

import numpy as np
import ml_dtypes

BF = ml_dtypes.bfloat16

B, OBS, ACT, E = 16384, 512, 64, 8
DIMS = [512, 1024, 512, 256, 64]
GH = 256
NCORES = 8
BSH = B // NCORES  # 2048
P = 128
FD = 512  # matmul free dim (one PSUM bank of f32)
NT = BSH // FD  # 4 n-tiles per core
NB = 2  # PSUM banks per group (drain op width = NB*FD)
NG = NT // NB  # groups per m-tile

KTS = [DIMS[l] // P for l in range(4)]  # [4, 8, 4, 2]
MTS = [DIMS[l + 1] // P for l in range(3)]  # [8, 4, 2]
W0_W = KTS[0] * DIMS[1]  # 4096
WA_W = W0_W + KTS[1] * DIMS[2]  # 8192
WB_W = KTS[2] * DIMS[3] + KTS[3] * DIMS[4]  # 1152
_BOFF = [0, 8, 12]  # bias blob col offsets for B0, B1, B2

_cache = {}


def _build(reps=1, dbg=False):
    """Build the Bass graph. reps>1 wraps the whole body in an outer For_i
    (the body is idempotent) — used only for timing via wall-time slope."""
    import concourse.bass as bass  # noqa: F401
    from concourse import bacc, mybir
    from concourse.bass import ds
    import concourse.tile as tile

    f32 = mybir.dt.float32
    bf16 = mybir.dt.bfloat16
    AF = mybir.ActivationFunctionType
    Alu = mybir.AluOpType

    nc = bacc.Bacc(None, target_bir_lowering=False)

    xTd = nc.dram_tensor("xT", [OBS, BSH], bf16, kind="ExternalInput")
    WAd = nc.dram_tensor("WA", [E + 1, P, WA_W], bf16, kind="ExternalInput")
    WBd = nc.dram_tensor("WB", [E + 1, P, WB_W], bf16, kind="ExternalInput")
    BIASd = nc.dram_tensor("BIAS", [E + 1, P, 16], f32, kind="ExternalInput")
    gW0d = nc.dram_tensor("gW0", [OBS, GH], bf16, kind="ExternalInput")
    gW1d = nc.dram_tensor("gW1", [GH, E], bf16, kind="ExternalInput")
    B3d = nc.dram_tensor("B3", [E, ACT], bf16, kind="ExternalInput")
    # packed small constants: gb0 (2 k-major cols) | gb1eff (col 2, rows 0:E)
    GSMd = nc.dram_tensor("GSM", [P, 3], f32, kind="ExternalInput")
    outd = nc.dram_tensor("out", [ACT, BSH], f32, kind="ExternalOutput")
    if dbg:
        dbg_wT = nc.dram_tensor("dbg_wT", [E, BSH], f32, kind="ExternalOutput")
        dbg_h1 = nc.dram_tensor("dbg_h1", [P, BSH], f32, kind="ExternalOutput")
        dbg_rw = nc.dram_tensor("dbg_rw", [P, BSH], f32, kind="ExternalOutput")
        dbg_h3s = nc.dram_tensor("dbg_h3s", [P, BSH], f32, kind="ExternalOutput")

    with tile.TileContext(nc) as tc:
        with (
            tc.tile_pool(name="const", bufs=1) as cpool,
            tc.tile_pool(name="epool", bufs=4) as epool,
            tc.tile_pool(name="rpool", bufs=4) as rpool,
            tc.tile_pool(name="psum", bufs=4, space="PSUM") as pspool,
            tc.tile_pool(name="dram", bufs=1, space="DRAM") as dpool,
        ):

            def body():
                # ---- static tiles ----
                gw0 = cpool.tile([P, OBS // P, GH], bf16, tag="gw0", name="gw0")
                nc.sync.dma_start(gw0[:], gW0d[:].rearrange("(ko p) o -> p ko o", p=P))
                xt = cpool.tile([P, OBS // P, BSH], bf16, tag="xt", name="xt")
                xt_src = xTd[:].rearrange("(ko p) n -> p ko n", p=P)
                for ko in range(OBS // P):
                    q = nc.sync if ko == 2 else nc.scalar
                    q.dma_start(xt[:, ko : ko + 1, :], xt_src[:, ko : ko + 1, :])
                gsm = cpool.tile([P, 3], f32, tag="gsm", name="gsm")
                nc.scalar.dma_start(gsm[:], GSMd[:])
                gw1 = cpool.tile([P, GH // P, E], bf16, tag="gw1", name="gw1")
                b3t = cpool.tile([E, ACT], bf16, tag="b3t", name="b3t")
                nc.scalar.dma_start(b3t[:], B3d[:])
                gb0t = gsm[:, 0:2]
                gb1t = gsm[0:E, 2:3]

                # expert weight buffers (static addresses: ldweights cannot
                # take register offsets) — refilled each iteration by
                # dynamic-source DMA
                wa0 = cpool.tile([P, KTS[0], DIMS[1]], bf16, tag="wa0", name="wa0")
                wa1 = cpool.tile([P, KTS[1], DIMS[2]], bf16, tag="wa1", name="wa1")
                wb2 = cpool.tile([P, KTS[2], DIMS[3]], bf16, tag="wb2", name="wb2")
                wb3 = cpool.tile([P, KTS[3], DIMS[4]], bf16, tag="wb3", name="wb3")
                bt = cpool.tile([P, 16], f32, tag="bias", name="bias")

                h1 = cpool.tile([P, MTS[0], BSH], bf16, tag="h1", name="h1")
                h2 = cpool.tile([P, MTS[1], BSH], bf16, tag="h2", name="h2")
                h3 = cpool.tile([P, MTS[2], BSH], bf16, tag="h3", name="h3")
                h3s = cpool.tile([P, MTS[2], BSH], bf16, tag="h3s", name="h3s")
                rw = cpool.tile([P, BSH], bf16, tag="rw", name="rw")
                acc = cpool.tile([ACT, BSH], f32, tag="acc", name="acc")

                def elu_wide(ps_flat, bias_ap, out_ap, mp=P):
                    # elu(z+b)+1 = min(exp(z+b), 1) + relu(z+b)
                    et = epool.tile([P, NB * FD], bf16, tag="e", name="e")[:mp]
                    nc.scalar.activation(et, ps_flat, AF.Exp, bias=bias_ap)
                    rt = rpool.tile([P, NB * FD], bf16, tag="r", name="r")[:mp]
                    nc.scalar.activation(rt, ps_flat, AF.Relu, bias=bias_ap)
                    nc.vector.scalar_tensor_tensor(
                        out_ap, et, 1.0, rt, Alu.min, Alu.add
                    )

                def psum_mm_groups(win_col, rhs_tile, KT, mp=P):
                    """k-outer over all NG*NB banks of one m-tile: each weight
                    load serves NT consecutive matmuls."""
                    psts = [
                        pspool.tile([P, NB, FD], f32, tag="ps", name="ps")
                        for _ in range(NG)
                    ]
                    for k in range(KT):
                        lhs = win_col(k)
                        for g in range(NG):
                            for n in range(NB):
                                ng = g * NB + n
                                nc.tensor.matmul(
                                    psts[g][:mp, n, :],
                                    lhs,
                                    rhs_tile[:, k, ng * FD : (ng + 1) * FD],
                                    start=(k == 0),
                                    stop=(k == KT - 1),
                                )
                    return [pst[:mp].rearrange("p a b -> p (a b)") for pst in psts]

                def layer(win, bt_sl, KT, MT, rhs_tile, out_tile):
                    for m in range(MT):
                        flats = psum_mm_groups(
                            lambda k, m=m: win[:, k, m * P : (m + 1) * P],
                            rhs_tile,
                            KT,
                        )
                        for g in range(NG):
                            elu_wide(
                                flats[g],
                                bt_sl[:, m : m + 1],
                                out_tile[:, m, g * NB * FD : (g + 1) * NB * FD],
                            )

                # ---- gate (prologue) ----
                def emit_gate():
                    # layer 1 (512 -> 256, elu'), k-outer across both m-tiles
                    # so each xt k-slice is consumed in one burst as it lands
                    gp = cpool.tile([P, GH // P, BSH], bf16, tag="gp", name="gp")
                    MT = GH // P
                    KT = OBS // P
                    psts = [
                        pspool.tile([P, NB, FD], f32, tag="ps", name="ps")
                        for _ in range(MT * NG)
                    ]
                    for k in range(KT):
                        for m in range(MT):
                            lhs = gw0[:, k, m * P : (m + 1) * P]
                            for g in range(NG):
                                for n in range(NB):
                                    ng = g * NB + n
                                    nc.tensor.matmul(
                                        psts[m * NG + g][:, n, :],
                                        lhs,
                                        xt[:, k, ng * FD : (ng + 1) * FD],
                                        start=(k == 0),
                                        stop=(k == KT - 1),
                                    )
                    for m in range(MT):
                        for g in range(NG):
                            elu_wide(
                                psts[m * NG + g].rearrange("p a b -> p (a b)"),
                                gb0t[:, m : m + 1],
                                gp[:, m, g * NB * FD : (g + 1) * NB * FD],
                            )
                    # layer 2 (256 -> 8) + exp
                    expT = cpool.tile([E, BSH], f32, tag="expT", name="expT")
                    gflats = psum_mm_groups(lambda k: gw1[:, k, :], gp, GH // P, mp=E)
                    for g in range(NG):
                        nc.scalar.activation(
                            expT[:, g * NB * FD : (g + 1) * NB * FD],
                            gflats[g],
                            AF.Exp,
                            bias=gb1t[:, 0:1],
                        )
                    # softmax denom via ones-matmul; reciprocal; wT = expT/sum
                    ones = cpool.tile([E, 1], f32, tag="ones", name="ones")
                    nc.vector.memset(ones[:], 1.0)
                    invs = cpool.tile([1, BSH], f32, tag="invs", name="invs")
                    sflats = psum_mm_groups(lambda k: ones[:], expT[:, None, :], 1, mp=1)
                    for g in range(NG):
                        nc.vector.reciprocal(
                            invs[:, g * NB * FD : (g + 1) * NB * FD], sflats[g]
                        )
                    inv_d = dpool.tile([1, BSH], f32, name="inv_d")
                    nc.scalar.dma_start(inv_d[:], invs[:])
                    rep8 = cpool.tile([E, BSH], f32, tag="rep8", name="rep8")
                    nc.scalar.dma_start(rep8[:], inv_d[0:1, :].to_broadcast((E, BSH)))
                    wT = cpool.tile([E, BSH], bf16, tag="wT", name="wT")
                    nc.vector.tensor_tensor(wT[:], expT[:], rep8[:], Alu.mult)
                    wt_d = dpool.tile([E, BSH], bf16, name="wt_d")
                    nc.scalar.dma_start(wt_d[:], wT[:])
                    return wT, wt_d

                # expert 0 weights
                nc.sync.dma_start(
                    wa0[:].rearrange("p k o -> p (k o)"), WAd[0][:, 0:W0_W]
                )
                nc.sync.dma_start(gw1[:], gW1d[:].rearrange("(ko p) o -> p ko o", p=P))
                nc.sync.dma_start(
                    wa1[:].rearrange("p k o -> p (k o)"), WAd[0][:, W0_W:WA_W]
                )
                nc.sync.dma_start(
                    wb2[:].rearrange("p k o -> p (k o)"),
                    WBd[0][:, 0 : KTS[2] * DIMS[3]],
                )
                nc.sync.dma_start(
                    wb3[:].rearrange("p k o -> p (k o)"),
                    WBd[0][:, KTS[2] * DIMS[3] : WB_W],
                )
                nc.scalar.dma_start(bt[:], BIASd[0])

                wT, wt_d = emit_gate()

                # acc = sum_e w[e] * b3eff_e  (K = E partitions)
                bflats = psum_mm_groups(lambda k: b3t[:, :], wT[:, None, :], 1, mp=ACT)
                for g in range(NG):
                    nc.scalar.activation(
                        acc[:, g * NB * FD : (g + 1) * NB * FD],
                        bflats[g],
                        AF.Copy,
                    )

                # ---- expert loop ----
                with tc.For_i(0, E, 1) as e:
                    # this expert's gate row, replicated over partitions
                    nc.scalar.dma_start(
                        rw[:], wt_d[ds(e, 1), :].to_broadcast((P, BSH))
                    )
                    layer(wa0, bt[:, _BOFF[0] : _BOFF[0] + MTS[0]], KTS[0], MTS[0], xt, h1)
                    nc.sync.dma_start(
                        wa0[:].rearrange("p k o -> p (k o)"),
                        WAd[ds(e + 1, 1)][0][:, 0:W0_W],
                    )
                    layer(wa1, bt[:, _BOFF[1] : _BOFF[1] + MTS[1]], KTS[1], MTS[1], h1, h2)
                    nc.sync.dma_start(
                        wa1[:].rearrange("p k o -> p (k o)"),
                        WAd[ds(e + 1, 1)][0][:, W0_W:WA_W],
                    )
                    layer(wb2, bt[:, _BOFF[2] : _BOFF[2] + MTS[2]], KTS[2], MTS[2], h2, h3)
                    nc.sync.dma_start(
                        wb2[:].rearrange("p k o -> p (k o)"),
                        WBd[ds(e + 1, 1)][0][:, 0 : KTS[2] * DIMS[3]],
                    )
                    nc.scalar.dma_start(bt[:], BIASd[ds(e + 1, 1)][0])
                    # h3s = h3 * w[e]
                    for k in range(KTS[3]):
                        nc.vector.tensor_tensor(
                            h3s[:, k, :], h3[:, k, :], rw[:], Alu.mult
                        )
                    # L3 (256 -> 64) accumulated into PSUM then acc
                    ps3 = [
                        pspool.tile([P, NB, FD], f32, tag="ps", name="ps")
                        for _ in range(NG)
                    ]
                    for k in range(KTS[3]):
                        for g in range(NG):
                            for n in range(NB):
                                ng = g * NB + n
                                nc.tensor.matmul(
                                    ps3[g][0:ACT, n, :],
                                    wb3[:, k, :],
                                    h3s[:, k, ng * FD : (ng + 1) * FD],
                                    start=(k == 0),
                                    stop=(k == KTS[3] - 1),
                                )
                    # prefetch W3(e+1) only after this iteration's L3 read wb3
                    nc.sync.dma_start(
                        wb3[:].rearrange("p k o -> p (k o)"),
                        WBd[ds(e + 1, 1)][0][:, KTS[2] * DIMS[3] : WB_W],
                    )
                    for g in range(NG):
                        gs = slice(g * NB * FD, (g + 1) * NB * FD)
                        nc.vector.tensor_tensor(
                            acc[:, gs],
                            acc[:, gs],
                            ps3[g][0:ACT].rearrange("p a b -> p (a b)"),
                            Alu.add,
                        )

                nc.scalar.dma_start(outd[:], acc[:])
                if dbg:
                    cvt = cpool.tile([P, BSH], f32, tag="cvt", name="cvt")
                    nc.vector.tensor_copy(cvt[0:E, :], wT[:])
                    nc.scalar.dma_start(dbg_wT[:], cvt[0:E, :])
                    nc.vector.tensor_copy(cvt[:], h1[:, 0, :])
                    nc.scalar.dma_start(dbg_h1[:], cvt[:])
                    nc.vector.tensor_copy(cvt[:], rw[:])
                    nc.scalar.dma_start(dbg_rw[:], cvt[:])
                    nc.vector.tensor_copy(cvt[:], h3s[:, 0, :])
                    nc.scalar.dma_start(dbg_h3s[:], cvt[:])

            if reps == 1:
                body()
            else:
                with tc.For_i(0, reps, 1):
                    body()

    nc.compile()
    return nc


def _prep_inputs(inputs):
    """Host-side: shard/transposes/casts + bias folding. Returns in_maps."""
    x = np.asarray(inputs["x"], np.float32)
    Ws = [np.asarray(inputs[f"W{l}"], np.float32) for l in range(4)]
    bs = [np.asarray(inputs[f"b{l}"], np.float32) for l in range(4)]
    gW0 = np.asarray(inputs["gW0"], np.float32)
    gb0 = np.asarray(inputs["gb0"], np.float32)
    gW1 = np.asarray(inputs["gW1"], np.float32)
    gb1 = np.asarray(inputs["gb1"], np.float32)

    shared = {}

    def kmajor(w, l):
        # [in, out] -> [P, KT*out]
        return (
            w.reshape(KTS[l], P, DIMS[l + 1]).transpose(1, 0, 2).reshape(P, -1)
        )

    wa = np.zeros((E + 1, P, WA_W), np.float32)
    wb = np.zeros((E + 1, P, WB_W), np.float32)
    for e in range(E):
        wa[e, :, 0:W0_W] = kmajor(Ws[0][e], 0)
        wa[e, :, W0_W:WA_W] = kmajor(Ws[1][e], 1)
        wb[e, :, 0 : KTS[2] * DIMS[3]] = kmajor(Ws[2][e], 2)
        wb[e, :, KTS[2] * DIMS[3] : WB_W] = kmajor(Ws[3][e], 3)
    shared["WA"] = np.ascontiguousarray(wa.astype(BF))
    shared["WB"] = np.ascontiguousarray(wb.astype(BF))

    # effective biases: layer l>0 consumes h' = elu+1, so subtract colsum(W_l)
    beff = [bs[0]] + [bs[l] - Ws[l].sum(axis=1) for l in range(1, 4)]
    bias = np.zeros((E + 1, P, 16), np.float32)
    for l in range(3):
        pk = beff[l].reshape(E, MTS[l], P).transpose(0, 2, 1)
        bias[:E, :, _BOFF[l] : _BOFF[l] + MTS[l]] = pk
    shared["BIAS"] = np.ascontiguousarray(bias)
    shared["B3"] = np.ascontiguousarray(beff[3].astype(BF))
    shared["gW0"] = np.ascontiguousarray(gW0.astype(BF))
    shared["gW1"] = np.ascontiguousarray(gW1.astype(BF))
    gsm = np.zeros((P, 3), np.float32)
    gsm[:, 0:2] = gb0.reshape(GH // P, P).T
    gsm[0:E, 2] = gb1 - gW1.sum(axis=0)
    shared["GSM"] = np.ascontiguousarray(gsm)

    in_maps = []
    for c in range(NCORES):
        m = dict(shared)
        m["xT"] = np.ascontiguousarray(x[c * BSH : (c + 1) * BSH].T.astype(BF))
        in_maps.append(m)
    return in_maps


def kernel(**inputs):
    from concourse.bass_utils import run_bass_kernel_spmd

    if "nc" not in _cache:
        _cache["nc"] = _build()
    nc = _cache["nc"]
    in_maps = _prep_inputs(inputs)
    res = run_bass_kernel_spmd(nc, in_maps, core_ids=list(range(NCORES)))
    full = np.empty((B, ACT), np.float32)
    for c in range(NCORES):
        full[c * BSH : (c + 1) * BSH] = np.asarray(res.results[c]["out"]).T
    return full


# revision 25
# speedup vs baseline: 1.3627x; 1.3627x over previous
"""Trainium2 Bass kernel for ActorMoE (8 experts, dims 512->1024->512->256->64).

Strategy: data-parallel across 8 NeuronCores (2048 rows each), weights
replicated. On-device compute is feature-major (features on partitions,
batch on the free dim) so the stacked expert weights W_l[e] (shape
[in, out]) are directly the matmul lhsT and no transposes are needed.

ELU trick: h' = elu(z)+1 = min(exp(z+b), max(z+(b+1), 1)), computed as
  e = Exp(z + b)                       (ScalarE, bias fused)
  h' = min(e, max(z + (b+1), 1))       (one custom DVE op: ELU_P1_MOE)
The +1 shift is corrected by subtracting colsum(W_next) from the next
layer's bias on the host, so the math is exact.

Matmuls are emitted with same-weight run length 4 (both 2-bank PSUM groups
of an m-tile accumulate together). Expert layers are software-pipelined
(tick t: load(t+1), L0(t), L2(t-1), L1(t), L3pair at even t) so each
layer-boundary ELU drain is covered by other matmul work; the gate fills
the first boundary.

L3 (256->64) is emitted per expert PAIR with column-tiled matmuls: expert
2p writes PSUM partitions 0-63, expert 2p+1 partitions 64-127
(tile_position col groups) so the two streams execute concurrently on the
PE array — M=64 alone would leave half the array idle. Only the bank's
first matmul uses start=True (bank-wide has_written clear); the second
expert's first write relies on unset has_written bits = overwrite.
The weighted accumulation over experts lands in acc[128, BSH] (even
experts in partitions 0-63, odd in 64-127); the host adds the two halves.

Per-expert weights/biases are packed into 3 DMA transfers (W0|W1 blob,
W2|W3 blob, bias blob) — DMA enqueue instructions cost ~600ns of
sequencer time each, so fewer/bigger transfers shorten the kernel's
startup ramp.

Softmax gate: logits are small (|logit| < ~2) so exp without max-shift is
safe. Per-expert gate rows are replicated across partitions via broadcast
DMA (bounced through DRAM, since partition-broadcast needs a DRAM source).
"""

import sys

sys.path.insert(0, "/opt/trn_rl_repo")

import numpy as np
import ml_dtypes

BF = ml_dtypes.bfloat16

B, OBS, ACT, E = 16384, 512, 64, 8
DIMS = [512, 1024, 512, 256, 64]
GH = 256
NCORES = 8
BSH = B // NCORES  # 2048
P = 128
FD = 512  # matmul free dim (one PSUM bank of f32)
NT = BSH // FD  # 4 n-tiles per core
NB = 2  # PSUM banks per group (ELU op width = NB*FD)
NG = NT // NB  # groups per m-tile

KTS = [DIMS[l] // P for l in range(4)]  # [4, 8, 4, 2]
MTS = [DIMS[l + 1] // P for l in range(3)]  # [8, 4, 2]
# bias blob column layout: B0, B0p1, B1, B1p1, B2, B2p1
_BOFF = [0, 8, 16, 20, 24, 28]
WA_W = KTS[0] * DIMS[1] + KTS[1] * DIMS[2]  # 8192
WB_W = KTS[2] * DIMS[3] + KTS[3] * DIMS[4]  # 1152

_cache = {}


def _get_elu_op():
    """Custom DVE op: out = min(in1, max(in0 + s0, 1)).
    With in0 = z (PSUM), s0 = b+1 per-partition, in1 = exp(z+b) from ACT,
    this computes elu(z+b)+1 in a single DVE pass."""
    if "elu_op" in _cache:
        return _cache["elu_op"]
    from concourse.dve_ops import DveOp, OPS
    from concourse.dve_spec import Spec, Src0, Src1, C0, One, maxx, minn, lower
    from concourse.dve_uop import DveOpSpec

    spec = Spec(
        body=minn(Src1, maxx(Src0 + C0, One)),
        reference=lambda in0, in1, s0: np.minimum(
            in1, np.maximum(in0 + s0, 1.0)
        ),
    )
    shas = {}
    for ver in ("v3", "v4"):
        s = DveOpSpec(name="ELU_P1_MOE", opcode=0, uops=lower(spec, ver=ver), rd1_en=True)
        shas[ver] = s.sha(ver)
    op = DveOp("ELU_P1_MOE", spec, subdim=False, uops_sha=shas)
    OPS.append(op)
    # import-time lookup tables don't see post-import appends — patch them
    import concourse.dve_ops as dve_ops_mod

    dve_ops_mod.CUSTOM_DVE_SPECS[op.name] = op.spec
    dve_ops_mod._SUB_OPCODE_FOR_NAME[op.name] = (
        dve_ops_mod._CUSTOM_DVE_ROW_BASE + len(OPS) - 1
    )
    _cache["elu_op"] = op
    return op


def _build(reps=1, nb=NB):
    """Build the Bass graph. reps>1 wraps the whole body in a For_i loop
    (the body is idempotent) — used only for timing via wall-time slope."""
    import concourse.bass as bass  # noqa: F401
    from concourse import bacc, mybir
    import concourse.tile as tile

    NB = nb
    NG = NT // NB
    PS_BUFS = 8 // NB
    E_BUFS = PS_BUFS + 2

    f32 = mybir.dt.float32
    bf16 = mybir.dt.bfloat16
    AF = mybir.ActivationFunctionType
    Alu = mybir.AluOpType

    nc = bacc.Bacc(None, target_bir_lowering=False)

    xTd = nc.dram_tensor("xT", [OBS, BSH], bf16, kind="ExternalInput")
    WAd = nc.dram_tensor("WA", [E, P, WA_W], bf16, kind="ExternalInput")
    WBd = nc.dram_tensor("WB", [E, P, WB_W], bf16, kind="ExternalInput")
    BIASd = nc.dram_tensor("BIAS", [E, P, 32], f32, kind="ExternalInput")
    gW0d = nc.dram_tensor("gW0", [OBS, GH], bf16, kind="ExternalInput")
    gW1d = nc.dram_tensor("gW1", [GH, E], bf16, kind="ExternalInput")
    # packed small constants: gb0 | gb0+1 | b3 pairs | gb1
    GSMd = nc.dram_tensor("GSM", [P, 9], f32, kind="ExternalInput")
    # out = pairs 0-2 accumulator, out2 = last pair's weighted term
    # (host sums all four 64-partition halves)
    outd = nc.dram_tensor("out", [P, BSH], f32, kind="ExternalOutput")
    out2d = nc.dram_tensor("out2", [P, BSH], f32, kind="ExternalOutput")

    with tile.TileContext(nc) as tc:
        with (
            tc.tile_pool(name="const", bufs=1) as cpool,
            tc.tile_pool(name="wapool", bufs=2) as wapool,
            tc.tile_pool(name="wbpool", bufs=4) as wbpool,
            tc.tile_pool(name="bpool", bufs=4) as bpool,
            tc.tile_pool(name="rwpool", bufs=2) as rwpool,
            tc.tile_pool(name="hpool", bufs=1) as hpool,
            tc.tile_pool(name="h3pool", bufs=2) as h3pool,
            tc.tile_pool(name="epool", bufs=E_BUFS) as epool,
            tc.tile_pool(name="rpool", bufs=4) as rpool,
            tc.tile_pool(name="tpool", bufs=2) as tpool,
            tc.tile_pool(name="psum", bufs=PS_BUFS, space="PSUM") as pspool,
            tc.tile_pool(name="dram", bufs=1, space="DRAM") as dpool,
        ):

            def body():
                # ---- load x and gate params ----
                # xt slices split across BOTH HW rings (scalar: k=0,1,3;
                # sync: k=2 ahead of the WA blob) so the gate's k-outer burn
                # rate matches slice arrivals and expert 0's weights land
                # right as the gate finishes — zero-gap handoff.
                gw0 = cpool.tile([P, OBS // P, GH], bf16, tag="gw0", name="gw0")
                nc.sync.dma_start(gw0[:], gW0d[:].rearrange("(ko p) o -> p ko o", p=P))
                xt = cpool.tile([P, OBS // P, BSH], bf16, tag="xt", name="xt")
                xt_src = xTd[:].rearrange("(ko p) n -> p ko n", p=P)
                for ko in range(OBS // P):
                    q = nc.sync if ko == 2 else nc.scalar
                    q.dma_start(xt[:, ko : ko + 1, :], xt_src[:, ko : ko + 1, :])
                gsm = cpool.tile([P, 9], f32, tag="gsm", name="gsm")
                nc.scalar.dma_start(gsm[:], GSMd[:])
                gw1 = cpool.tile([P, GH // P, E], bf16, tag="gw1", name="gw1")
                gb0t = gsm[:, 0:2]
                gb0p1t = gsm[:, 2:4]
                b3pt = gsm[:, 4:8]
                gb1t = gsm[0:E, 8:9]

                import os

                elu_mode = os.environ.get("ELU_MODE", "actx2")
                elu_op = _get_elu_op() if elu_mode == "cdve" else None

                def elu_wide(ps_flat, bias_ap, biasp1_ap, out_ap, mp=P):
                    # ps_flat: [mp, NB*FD] PSUM view.
                    if elu_mode == "cdve":
                        # one wide ACT + one custom DVE from PSUM:
                        # h' = min(exp(z+b), max(z+(b+1), 1)) = elu(z+b)+1
                        et = epool.tile([P, NB * FD], bf16, tag="e", name="e")[:mp]
                        nc.scalar.activation(et, ps_flat, AF.Exp, bias=bias_ap)
                        nc.vector._custom_dve(
                            elu_op, out=out_ap, in0=ps_flat, in1=et, s0=biasp1_ap
                        )
                        return
                    # elu(z+b)+1 = min(exp(z+b), 1) + relu(z+b): two ACT passes
                    # over PSUM (released early) + one all-bf16 DVE merge
                    et = epool.tile([P, NB * FD], bf16, tag="e", name="e")[:mp]
                    nc.scalar.activation(et, ps_flat, AF.Exp, bias=bias_ap)
                    rt = rpool.tile([P, NB * FD], bf16, tag="r", name="r")[:mp]
                    nc.scalar.activation(rt, ps_flat, AF.Relu, bias=bias_ap)
                    nc.vector.scalar_tensor_tensor(
                        out_ap, et, 1.0, rt, Alu.min, Alu.add
                    )

                def psum_mm_groups(win_col, rhs_tile, KT, mp=P):
                    """All NG groups of one m-tile accumulated together so each
                    weight load serves NT consecutive matmuls (same-weight run
                    length 4). Returns one flat [mp, NB*FD] view per group."""
                    psts = [
                        pspool.tile([P, NB, FD], f32, tag="ps", name="ps")
                        for _ in range(NG)
                    ]
                    for k in range(KT):
                        lhs = win_col(k)
                        for g in range(NG):
                            for n in range(NB):
                                ng = g * NB + n
                                nc.tensor.matmul(
                                    psts[g][:mp, n, :],
                                    lhs,
                                    rhs_tile[:, k, ng * FD : (ng + 1) * FD],
                                    start=(k == 0),
                                    stop=(k == KT - 1),
                                )
                    return [pst[:mp].rearrange("p a b -> p (a b)") for pst in psts]

                def layer(win, bt, btp1, KT, MT, rhs_tile, out_tile):
                    """z = win.T @ rhs + b; out = elu(z)+1 (bf16)."""
                    for m in range(MT):
                        flats = psum_mm_groups(
                            lambda k, m=m: win[:, k, m * P : (m + 1) * P],
                            rhs_tile,
                            KT,
                        )
                        for g in range(NG):
                            elu_wide(
                                flats[g],
                                bt[:, m : m + 1],
                                btp1[:, m : m + 1],
                                out_tile[:, m, g * NB * FD : (g + 1) * NB * FD],
                            )

                def emit_gate_l1():
                    # gate layer 1 (512 -> 256, elu'), k-outer across BOTH
                    # m-tiles (4 PSUM groups): each xt k-slice is consumed in
                    # one 8-MM burst so the matmuls keep pace with the
                    # k-sliced xt DMA arrivals at kernel start.
                    gp = cpool.tile([P, GH // P, BSH], bf16, tag="gp", name="gp")
                    MT = GH // P
                    KT = OBS // P
                    psts = [
                        pspool.tile([P, NB, FD], f32, tag="ps", name="ps")
                        for _ in range(MT * NG)
                    ]
                    for k in range(KT):
                        for m in range(MT):
                            lhs = gw0[:, k, m * P : (m + 1) * P]
                            for g in range(NG):
                                for n in range(NB):
                                    ng = g * NB + n
                                    nc.tensor.matmul(
                                        psts[m * NG + g][:, n, :],
                                        lhs,
                                        xt[:, k, ng * FD : (ng + 1) * FD],
                                        start=(k == 0),
                                        stop=(k == KT - 1),
                                    )
                    for m in range(MT):
                        for g in range(NG):
                            elu_wide(
                                psts[m * NG + g].rearrange("p a b -> p (a b)"),
                                gb0t[:, m : m + 1],
                                gb0p1t[:, m : m + 1],
                                gp[:, m, g * NB * FD : (g + 1) * NB * FD],
                            )
                    return gp

                def emit_gate_rest(gp):
                    # gate layer 2 (256 -> 8) + exp
                    expT = cpool.tile([E, BSH], f32, tag="expT", name="expT")
                    gflats = psum_mm_groups(lambda k: gw1[:, k, :], gp, GH // P, mp=E)
                    for g in range(NG):
                        nc.scalar.activation(
                            expT[:, g * NB * FD : (g + 1) * NB * FD],
                            gflats[g],
                            AF.Exp,
                            bias=gb1t[:, 0:1],
                        )
                    # softmax denom: sum over 8 experts via ones-matmul
                    ones = cpool.tile([E, 1], f32, tag="ones", name="ones")
                    nc.vector.memset(ones[:], 1.0)
                    invs = cpool.tile([1, BSH], f32, tag="invs", name="invs")
                    sflats = psum_mm_groups(lambda k: ones[:], expT[:, None, :], 1, mp=1)
                    for g in range(NG):
                        nc.vector.reciprocal(
                            invs[:, g * NB * FD : (g + 1) * NB * FD], sflats[g]
                        )
                    # wT[e, s] = exp(logit_e)/sum (partition-broadcast DMA
                    # needs a DRAM source, so bounce via DRAM)
                    inv_d = dpool.tile([1, BSH], f32, name="inv_d")
                    nc.scalar.dma_start(inv_d[:], invs[:])
                    rep8 = cpool.tile([E, BSH], f32, tag="rep8", name="rep8")
                    nc.scalar.dma_start(
                        rep8[:], inv_d[0:1, :].to_broadcast((E, BSH))
                    )
                    wT = cpool.tile([E, BSH], bf16, tag="wT", name="wT")
                    nc.vector.tensor_tensor(wT[:], expT[:], rep8[:], Alu.mult)
                    wt_d = dpool.tile([E, BSH], bf16, name="wt_d")
                    nc.scalar.dma_start(wt_d[:], wT[:])
                    return wt_d

                def load_expert(e):
                    st = {}
                    wa = wapool.tile([P, WA_W], bf16, tag="wa", name="wa")
                    # W0 half first as its own transfer: L0 only needs W0,
                    # so expert 0's first layer starts ~3us sooner at kernel
                    # start / rep boundary (W1 follows before L1 needs it).
                    w0w = KTS[0] * DIMS[1]
                    nc.sync.dma_start(wa[:, 0:w0w], WAd[e][:, 0:w0w])
                    nc.sync.dma_start(wa[:, w0w:WA_W], WAd[e][:, w0w:WA_W])
                    wb = wbpool.tile([P, WB_W], bf16, tag="wb", name="wb")
                    nc.sync.dma_start(wb[:], WBd[e])
                    bt = bpool.tile([P, 32], f32, tag="bias", name="bias")
                    nc.scalar.dma_start(bt[:], BIASd[e])
                    off = KTS[0] * DIMS[1]
                    st["w0"] = wa[:, 0:off].rearrange("p (k o) -> p k o", k=KTS[0])
                    st["w1"] = wa[:, off:WA_W].rearrange("p (k o) -> p k o", k=KTS[1])
                    off = KTS[2] * DIMS[3]
                    st["w2"] = wb[:, 0:off].rearrange("p (k o) -> p k o", k=KTS[2])
                    st["w3"] = wb[:, off:WB_W].rearrange("p (k o) -> p k o", k=KTS[3])
                    st["bts"] = [
                        (
                            bt[:, _BOFF[2 * l] : _BOFF[2 * l] + MTS[l]],
                            bt[:, _BOFF[2 * l + 1] : _BOFF[2 * l + 1] + MTS[l]],
                        )
                        for l in range(3)
                    ]
                    return st

                def emit_L0(st):
                    st["h1"] = hpool.tile(
                        [P, DIMS[1] // P, BSH], bf16, tag="h1", name="h1"
                    )
                    layer(
                        st["w0"], st["bts"][0][0], st["bts"][0][1],
                        KTS[0], DIMS[1] // P, xt, st["h1"],
                    )

                def emit_L1(st):
                    st["h2"] = hpool.tile(
                        [P, DIMS[2] // P, BSH], bf16, tag="h2", name="h2"
                    )
                    layer(
                        st["w1"], st["bts"][1][0], st["bts"][1][1],
                        KTS[1], DIMS[2] // P, st["h1"], st["h2"],
                    )

                def emit_L2(st):
                    st["h3"] = h3pool.tile(
                        [P, DIMS[3] // P, BSH], bf16, tag="h3", name="h3"
                    )
                    layer(
                        st["w2"], st["bts"][2][0], st["bts"][2][1],
                        KTS[2], DIMS[3] // P, st["h2"], st["h3"],
                    )

                def emit_rw_pair(pair, wt_d):
                    """Gate rows for experts (2p, 2p+1) replicated over the
                    two 64-partition halves; prefetched a tick early."""
                    eA, eB = 2 * pair, 2 * pair + 1
                    rw = rwpool.tile([P, BSH], bf16, tag="rw", name="rw")
                    nc.scalar.dma_start(
                        rw[0:ACT, :], wt_d[eA : eA + 1, :].to_broadcast((ACT, BSH))
                    )
                    nc.scalar.dma_start(
                        rw[ACT:P, :], wt_d[eB : eB + 1, :].to_broadcast((ACT, BSH))
                    )
                    return rw

                def emit_L3_pair(stA, stB, pair, acc, rw):
                    """L3 (256 -> 64) for experts (2p, 2p+1), column-tiled:
                    expert A -> PSUM partitions 0-63, B -> 64-127 so the two
                    matmul streams execute concurrently on the PE array.
                    acc[0:64] accumulates even experts, acc[64:128] odd."""
                    psts = [
                        pspool.tile([P, NB, FD], f32, tag="ps", name="ps")
                        for _ in range(NG)
                    ]
                    KT = KTS[3]
                    # Per bank: A's full k-accumulation first, then B's with
                    # its own start=True — the bank-wide has_written clear
                    # doesn't erase A's finished data and nothing rewrites
                    # A's partitions after. Adjacent banks interleave A/B so
                    # the two col-groups still overlap on the array.
                    for g in range(NG):
                        for n in range(NB):
                            ng = g * NB + n
                            rs = slice(ng * FD, (ng + 1) * FD)
                            for st_, base in ((stA, 0), (stB, ACT)):
                                for k in range(KT):
                                    nc.tensor.matmul(
                                        psts[g][base : base + ACT, n, :],
                                        st_["w3"][:, k, :],
                                        st_["h3"][:, k, rs],
                                        start=(k == 0),
                                        stop=(k == KT - 1),
                                        skip_group_check=True,
                                    )
                    for g in range(NG):
                        gs = slice(g * NB * FD, (g + 1) * NB * FD)
                        ps_flat = psts[g].rearrange("p a b -> p (a b)")
                        if pair == 0:
                            nc.vector.scalar_tensor_tensor(
                                acc[:, gs], ps_flat, b3pt[:, pair : pair + 1],
                                rw[:, gs], Alu.add, Alu.mult,
                            )
                        else:
                            tt = tpool.tile([P, NB * FD], f32, tag="t", name="t")
                            nc.vector.scalar_tensor_tensor(
                                tt, ps_flat, b3pt[:, pair : pair + 1],
                                rw[:, gs], Alu.add, Alu.mult,
                            )
                            if pair == E // 2 - 1:
                                # last pair: skip the accumulate — its term
                                # streams out on its own tensor (host sums),
                                # shortening the end-of-kernel DVE drain.
                                nc.scalar.dma_start(out2d[:, gs], tt[:])
                            else:
                                nc.vector.tensor_add(acc[:, gs], acc[:, gs], tt)

                acc = cpool.tile([P, BSH], f32, tag="acc", name="acc")

                # software pipeline over ticks t=0..E; L3 per expert pair at
                # even t covers the (t-2, t-1) experts whose h3 are both ready.
                state = {0: load_expert(0)}
                # gw1 is small and first needed at gate_rest — load it after
                # expert 0's blobs so they don't wait behind it on the ring.
                nc.sync.dma_start(gw1[:], gW1d[:].rearrange("(ko p) o -> p ko o", p=P))
                wt_d = None
                rw = None
                for t in range(E + 1):
                    if t == 0:
                        gp = emit_gate_l1()
                    if t < E:
                        if t + 1 < E:
                            state[t + 1] = load_expert(t + 1)
                        emit_L0(state[t])
                    if t == 0:
                        wt_d = emit_gate_rest(gp)
                    if t % 2 == 1:
                        # prefetch next pair's gate rows a tick early
                        rw = emit_rw_pair((t - 1) // 2, wt_d)
                    if t >= 1:
                        emit_L2(state[t - 1])
                    if t < E:
                        emit_L1(state[t])
                    if t == E:
                        # acc is final after pair 2 (tick 6) — stream it out
                        # under the last tick's compute.
                        nc.scalar.dma_start(outd[:], acc[:])
                    if t >= 2 and t % 2 == 0:
                        emit_L3_pair(
                            state[t - 2], state[t - 1], (t - 2) // 2, acc, rw
                        )
                        del state[t - 2], state[t - 1]

            if reps == 1:
                body()
            else:
                with tc.For_i(0, reps, 1):
                    body()

    nc.compile()
    return nc


def _prep_inputs(inputs):
    """Host-side: shard/transposes/casts + bias folding. Returns in_maps."""
    x = np.asarray(inputs["x"], np.float32)
    Ws = [np.asarray(inputs[f"W{l}"], np.float32) for l in range(4)]
    bs = [np.asarray(inputs[f"b{l}"], np.float32) for l in range(4)]
    gW0 = np.asarray(inputs["gW0"], np.float32)
    gb0 = np.asarray(inputs["gb0"], np.float32)
    gW1 = np.asarray(inputs["gW1"], np.float32)
    gb1 = np.asarray(inputs["gb1"], np.float32)

    shared = {}

    def kmajor(w, l):
        # [in, out] -> [P, KT, out] -> [P, KT*out]
        return (
            w.reshape(KTS[l], P, DIMS[l + 1]).transpose(1, 0, 2).reshape(P, -1)
        )

    wa = np.concatenate(
        [
            np.stack([kmajor(Ws[0][e], 0) for e in range(E)]),
            np.stack([kmajor(Ws[1][e], 1) for e in range(E)]),
        ],
        axis=2,
    )
    shared["WA"] = np.ascontiguousarray(wa.astype(BF))
    wb = np.concatenate(
        [
            np.stack([kmajor(Ws[2][e], 2) for e in range(E)]),
            np.stack([kmajor(Ws[3][e], 3) for e in range(E)]),
        ],
        axis=2,
    )
    shared["WB"] = np.ascontiguousarray(wb.astype(BF))

    # effective biases: layer l>0 consumes h' = elu+1, so subtract colsum(W_l)
    beff = [bs[0]] + [bs[l] - Ws[l].sum(axis=1) for l in range(1, 4)]
    bias = np.zeros((E, P, 32), np.float32)
    for l in range(3):
        pk = beff[l].reshape(E, MTS[l], P).transpose(0, 2, 1)
        bias[:, :, _BOFF[2 * l] : _BOFF[2 * l] + MTS[l]] = pk
        bias[:, :, _BOFF[2 * l + 1] : _BOFF[2 * l + 1] + MTS[l]] = pk + 1.0
    shared["BIAS"] = np.ascontiguousarray(bias)
    shared["gW0"] = np.ascontiguousarray(gW0.astype(BF))
    shared["gW1"] = np.ascontiguousarray(gW1.astype(BF))
    # packed small constants: gb0 | gb0+1 | b3 pairs | gb1
    gsm = np.zeros((P, 9), np.float32)
    gpk = gb0.reshape(GH // P, P).T
    gsm[:, 0:2] = gpk
    gsm[:, 2:4] = gpk + 1.0
    for p_ in range(E // 2):
        gsm[0:ACT, 4 + p_] = beff[3][2 * p_]
        gsm[ACT:P, 4 + p_] = beff[3][2 * p_ + 1]
    gsm[0:E, 8] = gb1 - gW1.sum(axis=0)
    shared["GSM"] = np.ascontiguousarray(gsm)

    in_maps = []
    for c in range(NCORES):
        m = dict(shared)
        m["xT"] = np.ascontiguousarray(x[c * BSH : (c + 1) * BSH].T.astype(BF))
        in_maps.append(m)
    return in_maps


def kernel(**inputs):
    from concourse.bass_utils import run_bass_kernel_spmd

    if "nc" not in _cache:
        _cache["nc"] = _build()
    nc = _cache["nc"]
    in_maps = _prep_inputs(inputs)
    res = run_bass_kernel_spmd(nc, in_maps, core_ids=list(range(NCORES)))
    full = np.empty((B, ACT), np.float32)
    for c in range(NCORES):
        o = np.asarray(res.results[c]["out"])
        o2 = np.asarray(res.results[c]["out2"])
        full[c * BSH : (c + 1) * BSH] = (
            o[0:ACT] + o[ACT:P] + o2[0:ACT] + o2[ACT:P]
        ).T
    return full

